# revision 20
# baseline (speedup 1.0000x reference)
"""Causal multi-head attention block (B=4, T=2048, C=1024, H=16, D=64) on 8 trn2 cores.

Sharding: core c -> (batch b = c//2, head-group g = c%2 covering heads 8g..8g+8).
Each core computes qkv projection for its batch restricted to its 8 heads,
flash-style causal attention in transposed orientation, and a partial output
projection; a pairwise fp16 ReduceScatter per 512-query block sums the two
head-group partials and scatters each core its 256-token half of y.

Pipeline structure: x is loaded in 512-column chunks so the first qk projection
starts ~8us in; only the n=0 qk chunk and the first 4 v tiles are emitted ahead
of attention. The remaining qkv projections and all output projections drain as
PE filler jobs inside the (ACT-bound) attention loop. Causal masking runs as
in-place affine_select on GpSimd; PSUM pv tiles are evacuated with a single
copy so normalization stays off the critical path. All matmuls fp16 at N=512.
"""
import sys

sys.path.insert(0, '/opt/trn_rl_repo')

from contextlib import ExitStack

import numpy as np

import concourse.bass as bass
import concourse.mybir as mybir
import concourse.tile as tile
from concourse import bacc
from concourse.bass_utils import run_bass_kernel_spmd

B, T, C = 4, 2048, 1024
H, D = 16, 64
HL = H // 2            # heads per core
NP = HL // 2           # head pairs per core
KC = C // 128          # contraction chunks for qkv projection
NT1 = T // 512         # 512-wide query blocks
NT2 = T // 128         # 128-tall key tiles
F32 = mybir.dt.float32
F16 = mybir.dt.float16
EXP = mybir.ActivationFunctionType.Exp
GROUPS = [[0, 1], [2, 3], [4, 5], [6, 7]]

_cached = {}


def install_profile_hook():
    """The agent image's antenv lacks axon_hooks; synthesize it so
    run_bass_kernel_spmd(trace=True) can capture NTFF profiles."""
    import types
    if 'antenv.axon_hooks' in sys.modules:
        return
    mod = types.ModuleType('antenv.axon_hooks')
    mod._hook = None

    def set_axon_ntff_profile_hook(h):
        mod._hook = h

    def get_axon_ntff_profile_hook():
        return mod._hook

    mod.set_axon_ntff_profile_hook = set_axon_ntff_profile_hook
    mod.get_axon_ntff_profile_hook = get_axon_ntff_profile_hook
    sys.modules['antenv.axon_hooks'] = mod
    try:
        from trn_agent_boot.trn_boot import _ntff_profile_via_ctypes
        set_axon_ntff_profile_hook(_ntff_profile_via_ctypes('/opt/axon/libaxon_pjrt.so'))
    except Exception as e:
        print(f"profile hook install failed: {e}", file=sys.stderr)


def build_kernel():
    if 'nc' in _cached:
        return _cached['nc']
    nc = bacc.Bacc("TRN2", target_bir_lowering=False, debug=False, num_devices=8)

    xT = nc.declare_dram_parameter("xT", [C, T], F16, isOutput=False)
    w_qk = nc.declare_dram_parameter("w_qk", [C, 2 * HL * D], F16, isOutput=False)
    w_v = nc.declare_dram_parameter("w_v", [C, HL * D], F16, isOutput=False)
    b_qk = nc.declare_dram_parameter("b_qk", [2 * HL * D, 1], F32, isOutput=False)
    b_v = nc.declare_dram_parameter("b_v", [1, HL * D], F16, isOutput=False)
    w_proj = nc.declare_dram_parameter("w_proj", [HL * D, C], F16, isOutput=False)
    b_proj_half = nc.declare_dram_parameter("b_proj_half", [1, C], F16, isOutput=False)
    y_rs = nc.declare_dram_parameter("y_rs", [NT1 * 256, C], F16, isOutput=True)

    with tile.TileContext(nc) as tc, ExitStack() as st:
        cpool = st.enter_context(tc.tile_pool(name="const", bufs=1))
        v_pool = st.enter_context(tc.tile_pool(name="vstore", bufs=1))
        qk_pool = st.enter_context(tc.tile_pool(name="qkT", bufs=1))
        o_pool = st.enter_context(tc.tile_pool(name="outT", bufs=1, side="right"))
        xpool = st.enter_context(tc.tile_pool(name="xT", bufs=1))
        wpool = st.enter_context(tc.tile_pool(name="wqk", bufs=1))
        wpp = st.enter_context(tc.tile_pool(name="wproj", bufs=1))
        ppool = st.enter_context(tc.tile_pool(name="ptile", bufs=4))
        npool = st.enter_context(tc.tile_pool(name="norm", bufs=4))
        ypool = st.enter_context(tc.tile_pool(name="ytile", bufs=4))
        mmps = st.enter_context(tc.tile_pool(name="mm_ps", bufs=2, space="PSUM"))
        sps = st.enter_context(tc.tile_pool(name="s_ps", bufs=1, space="PSUM"))
        pvps = st.enter_context(tc.tile_pool(name="pv_ps", bufs=1, space="PSUM"))
        dram = st.enter_context(tc.tile_pool(name="dram", bufs=1, space="DRAM"))

        # ---- ACT exp-table prewarm: first Scalar instruction, runs at t~0 ----
        warm = cpool.tile([1, 8], F32)
        nc.gpsimd.memset(warm[:], 0.0)
        warm2 = cpool.tile([1, 8], F16)
        nc.scalar.activation(warm2[:], warm[:], EXP, scale=1.0)

        # ---- constants ----
        ones128h = cpool.tile([1, 128], F16)
        nc.gpsimd.memset(ones128h[:], 1.0)
        ones_p = cpool.tile([128, HL], F16)
        nc.gpsimd.memset(ones_p[:], 1.0)
        bqk_sb = cpool.tile([128, 2 * NP, 1], F32)
        nc.sync.dma_start(bqk_sb[:], b_qk[:].rearrange("(c p) o -> p c o", p=128))
        bv_sb0 = cpool.tile([1, HL * D], F16)
        nc.sync.dma_start(bv_sb0[:], b_v[:])
        bp_sb = cpool.tile([1, C], F16)
        nc.sync.dma_start(bp_sb[:], b_proj_half[:])

        # ---- input DMAs, chunked so compute starts early ----
        xTt = [xpool.tile([128, T], F16, tag=f"x{kc}", name=f"x{kc}")
               for kc in range(KC)]
        wqk_sb = [wpool.tile([128, 2 * HL * D], F16, tag=f"w{kc}", name=f"w{kc}")
                  for kc in range(KC)]
        # critical loads first, interleaved so they spread across DMA queues
        for kc in range(KC):
            nc.sync.dma_start(xTt[kc][:, 0:512], xT[bass.ts(kc, 128), 0:512])
            nc.sync.dma_start(wqk_sb[kc][:], w_qk[bass.ts(kc, 128), :])
        wv_sb = [wpool.tile([128, HL * D], F16, tag=f"wv{kc}", name=f"wv{kc}")
                 for kc in range(KC)]
        for kc in range(KC):
            nc.sync.dma_start(wv_sb[kc][:], w_v[bass.ts(kc, 128), :])
        for kc in range(KC):
            nc.sync.dma_start(xTt[kc][:, bass.ds(512, 512)],
                              xT[bass.ts(kc, 128), bass.ds(512, 512)])
        wp_sb = [wpp.tile([128, C], F16, tag=f"wp{j}", name=f"wp{j}")
                 for j in range(NP)]

        def emit_late_dmas():
            # x n=2,3 + w_proj: needed only from blk1/blk2 on; issued after
            # the stage-A head so they don't compete with critical loads
            for n in range(2, NT1):
                for kc in range(KC):
                    nc.sync.dma_start(xTt[kc][:, bass.ts(n, 512)],
                                      xT[bass.ts(kc, 128), bass.ts(n, 512)])
            for j in range(NP):
                nc.sync.dma_start(wp_sb[j][:], w_proj[bass.ts(j, 128), :])

        # ---- bias broadcast tiles (emitted after the qk head so their
        # dependency on the tiny bias DMAs never gates the first matmuls) ----
        bvb = cpool.tile([128, HL, D], F32)
        bpb = cpool.tile([128, C], F32)

        def emit_bias_broadcasts():
            bvb_ps = mmps.tile([128, HL * D], F32, tag="mm")
            nc.tensor.matmul(bvb_ps[:], ones128h[:], bv_sb0[:],
                             start=True, stop=True)
            nc.vector.tensor_copy(
                bvb[:], bvb_ps[:].rearrange("p (h d) -> p h d", h=HL))
            for n in range(2):
                bpb_ps = mmps.tile([128, 512], F32, tag="mm")
                nc.tensor.matmul(bpb_ps[:], ones128h[:],
                                 bp_sb[:, bass.ts(n, 512)],
                                 start=True, stop=True)
                nc.vector.tensor_copy(bpb[:, bass.ts(n, 512)], bpb_ps[:])

        # persistent activation stores
        vst = [v_pool.tile([128, HL, D + 1], F16, tag=f"vs{m}", name=f"vs{m}")
               for m in range(NT2)]
        qkT = [qk_pool.tile([128, T], F16, tag=f"qk{j}", name=f"qk{j}")
               for j in range(2 * NP)]
        outT = [o_pool.tile([128, T], F16, tag=f"o{j}", name=f"o{j}")
                for j in range(NP)]

        # per-block y partials + RS outputs (fp16, internal DRAM)
        y_blk = [dram.tile([512, C], F16, tag=f"yb{b}", name=f"yb{b}")
                 for b in range(NT1)]
        rs_blk = [dram.tile([256, C], F16, tag=f"rb{b}", name=f"rb{b}")
                  for b in range(NT1)]

        oc_order = [oc for j in range(NP) for oc in (j, NP + j)]

        def emit_qk_half(oc, n, half, ps):
            # one 4-kc half of a qk projection; half=1 finishes with bias add
            for kc in range(4 * half, 4 * half + 4):
                nc.tensor.matmul(
                    ps[:], wqk_sb[kc][:, bass.ts(oc, 128)],
                    xTt[kc][:, bass.ts(n, 512)],
                    start=(kc == 0), stop=(kc == KC - 1))
            if half == 1:
                nc.vector.tensor_scalar_add(
                    qkT[oc][:, bass.ts(n, 512)], ps[:], bqk_sb[:, oc, :])

        def emit_qk_job(oc, n):
            ps = mmps.tile([128, 512], F32, tag="mm")
            emit_qk_half(oc, n, 0, ps)
            emit_qk_half(oc, n, 1, ps)

        def emit_v_job(m):
            ps = mmps.tile([128, HL * D], F32, tag="mm")
            for kc in range(KC):
                nc.tensor.matmul(
                    ps[:], xTt[kc][:, bass.ts(m, 128)], wv_sb[kc][:],
                    start=(kc == 0), stop=(kc == KC - 1))
            nc.vector.tensor_add(
                vst[m][:, :, 0:D],
                ps[:].rearrange("p (h d) -> p h d", h=HL), bvb[:])
            nc.vector.tensor_copy(vst[m][:, :, D], ones_p[:])

        def emit_proj_job(blk, mt, n):
            ps = mmps.tile([128, 512], F32, tag="mm")
            for j in range(NP):
                nc.tensor.matmul(
                    ps[:], outT[j][:, bass.ds(blk * 512 + mt * 128, 128)],
                    wp_sb[j][:, bass.ts(n, 512)],
                    start=(j == 0), stop=(j == NP - 1))
            yt = ypool.tile([128, 512], F16, tag="yt")
            nc.vector.tensor_add(yt[:], ps[:], bpb[:, bass.ts(n, 512)])
            nc.sync.dma_start(
                y_blk[blk][bass.ts(mt, 128), bass.ts(n, 512)], yt[:])

        def emit_rs(b):
            nc.gpsimd.collective_compute(
                "ReduceScatter", mybir.AluOpType.add,
                replica_groups=GROUPS,
                ins=[y_blk[b][:].opt()],
                outs=[rs_blk[b][:].opt()],
            )
            nc.sync.dma_start(y_rs[bass.ds(b * 256, 256), :], rs_blk[b][:])

        # ---- stage A head start: qk for n=0 (split in 4-kc halves so the
        # first matmuls only need the first half of x-n0/wqk), v for m=0..3 ----
        half_ps = {}
        head_seq = []
        for idx, oc in enumerate(oc_order):
            if idx >= 2:
                head_seq.append((oc_order[idx - 2], 1))
            head_seq.append((oc, 0))
        head_seq += [(oc_order[-2], 1), (oc_order[-1], 1)]
        for oc, half in head_seq:
            if half == 0:
                half_ps[oc] = mmps.tile([128, 512], F32, tag="mm", name=f"hps{oc}")
            emit_qk_half(oc, 0, half, half_ps[oc])
        emit_bias_broadcasts()
        for m in range(4):
            emit_v_job(m)
        emit_late_dmas()

        # filler job queue drained inside the attention loop (FIFO; order
        # guarantees qk(n)/v(m) land before the block that consumes them)
        pending = []
        for m in range(4, 8):
            pending.append(('v', (m,)))
        for oc in oc_order:
            pending.append(('qk', (oc, 1)))

        def drain(k):
            for _ in range(min(k, len(pending))):
                kind, args = pending.pop(0)
                if kind == 'qk':
                    emit_qk_job(*args)
                elif kind == 'v':
                    emit_v_job(*args)
                else:
                    emit_proj_job(*args)

        budgets = {0: 3, 1: 5, 2: 5, 3: 4}
        blk_tails = []

        def emit_tails():
            # normalize tails: deferred so the inter-pair DVE queue carries
            # only the PSUM-releasing copies (pv bank turnaround stays short)
            while blk_tails:
                tj, h, rs_sb, src_ap = blk_tails.pop(0)
                rec = npool.tile([1, 512], F32, tag="rec", bufs=3)
                nc.vector.reciprocal_approx_fast(rec[:], rs_sb[:])
                rec16 = npool.tile([1, 512], F16, tag="rec16", bufs=3)
                nc.vector.tensor_copy(rec16[:], rec[:])
                rb = npool.tile([64, 512], F16, tag="rb", bufs=3)
                nc.gpsimd.partition_broadcast(rb[:], rec16[:])
                nc.vector.tensor_mul(
                    outT[tj][h * 64:(h + 1) * 64,
                             bass.ds(blk * 512, 512)], src_ap, rb[:])
        # ---- attention + interleaved filler + per-block proj/RS ----
        for blk in range(NT1):
            t1 = bass.ds(blk * 512, 512)
            nt2 = 4 * (blk + 1)
            for j in range(NP):
                quota = budgets[blk]
                emitted = 0
                if blk == 3 and j == 2:
                    emit_rs(2)   # proj(2) fully drained by now
                q_t, k_t = qkT[j], qkT[NP + j]
                pv1 = pvps.tile([D + 1, 512], F32, tag="pvA", bufs=1)
                pv2 = pvps.tile([D + 1, 512], F32, tag="pvB", bufs=1)
                for i in range(nt2):
                    # spread filler jobs across iterations instead of lumping
                    # them ahead of the block (keeps ACT fed via sAB lookahead)
                    want = (quota * (i + 1) + nt2 - 1) // nt2
                    if want > emitted:
                        drain(want - emitted)
                        emitted = want
                    t2 = bass.ds(i * 128, 128)
                    sAB = sps.tile([128, 1024], F32, tag="sAB", bufs=2)
                    nc.tensor.matmul(sAB[:, 0:512], k_t[0:64, t2], q_t[0:64, t1],
                                     start=True, stop=True, tile_position=(0, 0))
                    nc.tensor.matmul(sAB[:, 512:1024], k_t[64:128, t2],
                                     q_t[64:128, t1],
                                     start=True, stop=True, tile_position=(64, 0))
                    pAB = ppool.tile([128, 2, 512], F16, tag="pAB", bufs=6)
                    nc.scalar.activation(
                        pAB[:].rearrange("p h q -> p (h q)"), sAB[:], EXP,
                        scale=0.125)
                    off = i * 128 - blk * 512
                    if off >= 0:
                        # only columns q < off+128 can fail q >= a+off (a<128)
                        w = min(off + 128, 512)
                        nc.gpsimd.affine_select(
                            out=pAB[:, :, 0:w], in_=pAB[:, :, 0:w],
                            compare_op=mybir.AluOpType.is_ge,
                            fill=0.0, base=-off, pattern=[[0, 2], [1, w]],
                            channel_multiplier=-1,
                        )
                    nc.tensor.matmul(pv1[:], vst[i][:, 2 * j, :],
                                     pAB[:, 0, :],
                                     start=(i == 0), stop=(i == nt2 - 1))
                    nc.tensor.matmul(pv2[:], vst[i][:, 2 * j + 1, :],
                                     pAB[:, 1, :],
                                     start=(i == 0), stop=(i == nt2 - 1))
                # evacuate both PSUM banks first (copies release them for the
                # next pair); normalize tails run after, off the critical path.
                # The very last pair skips the evacuation copy (banks are not
                # reused) and multiplies straight from PSUM.
                last = (blk == NT1 - 1 and j == NP - 1)
                for h, pv in ((0, pv1), (1, pv2)):
                    rs_sb = npool.tile([1, 512], F32, tag="rs_sb", bufs=9)
                    nc.vector.tensor_copy(rs_sb[:], pv[D:D + 1, :])
                    if last:
                        blk_tails.append((j, h, rs_sb, pv[0:D, :]))
                    else:
                        pvc = npool.tile([D, 512], F16, tag="pvc", bufs=9)
                        nc.vector.tensor_copy(pvc[:], pv[0:D, :])  # frees bank
                        blk_tails.append((j, h, rs_sb, pvc[:]))
                if blk == NT1 - 1:
                    emit_tails()
            if blk < NT1 - 1:
                emit_tails()
            # next block's inputs first, then this block's proj jobs (proj
            # waits on this block's normalize — keep it off the queue front)
            if blk < NT1 - 1:
                for m in range(4 * (blk + 2), 4 * (blk + 2) + 4):
                    if m < NT2:
                        pending.append(('v', (m,)))
                for oc in oc_order:
                    if blk + 2 < NT1:
                        pending.append(('qk', (oc, blk + 2)))
            for mt in range(4):
                for n in range(2):
                    pending.append(('proj', (blk, mt, n)))
            if blk == 1:
                emit_rs(0)       # proj(0) drained during blk1
            elif blk == 2:
                emit_rs(1)       # proj(1) drained during blk2
        while pending:
            drain(len(pending))
        emit_rs(NT1 - 1)

    nc.compile()
    _cached['nc'] = nc
    return nc


def make_in_maps(x, w_qkv, b_qkv, w_proj, b_proj):
    x = np.asarray(x, dtype=np.float32)
    w_qkv = np.asarray(w_qkv, dtype=np.float32)
    b_qkv = np.asarray(b_qkv, dtype=np.float32)
    w_proj = np.asarray(w_proj, dtype=np.float32)
    b_proj = np.asarray(b_proj, dtype=np.float32)

    in_maps = []
    for c in range(8):
        b, g = c // 2, c % 2
        heads = list(range(g * HL, (g + 1) * HL))
        # paired column order: chunk j = [q(h_{2j}) | q(h_{2j+1})], then k chunks
        qcols, kcols = [], []
        for j in range(NP):
            for h in (heads[2 * j], heads[2 * j + 1]):
                qcols.extend(range(h * D, (h + 1) * D))
                kcols.extend(range(C + h * D, C + (h + 1) * D))
        vcols = [2 * C + h * D + d for h in heads for d in range(D)]
        qk_idx = np.array(qcols + kcols)
        v_idx = np.array(vcols)
        p_idx = np.array([h * D + d for h in heads for d in range(D)])

        in_maps.append({
            "xT": np.ascontiguousarray(x[b].T.astype(np.float16)),
            "w_qk": np.ascontiguousarray(w_qkv[:, qk_idx].astype(np.float16)),
            "w_v": np.ascontiguousarray(w_qkv[:, v_idx].astype(np.float16)),
            "b_qk": np.ascontiguousarray(b_qkv[qk_idx][:, None]),
            "b_v": np.ascontiguousarray(b_qkv[v_idx][None, :].astype(np.float16)),
            "w_proj": np.ascontiguousarray(w_proj[p_idx, :].astype(np.float16)),
            "b_proj_half": np.ascontiguousarray(0.5 * b_proj[None, :].astype(np.float16)),
        })
    return in_maps


def run(inputs, trace=False):
    if trace:
        install_profile_hook()
    nc = build_kernel()
    in_maps = make_in_maps(**inputs)
    res = run_bass_kernel_spmd(nc, in_maps, list(range(8)), trace=trace)
    out = np.empty((B, T, C), dtype=np.float32)
    for c in range(8):
        b, g = c // 2, c % 2
        piece = res.results[c]["y_rs"].astype(np.float32)
        for q in range(NT1):
            out[b, q * 512 + g * 256: q * 512 + (g + 1) * 256, :] = \
                piece[q * 256:(q + 1) * 256]
    return out, res


def kernel(**inputs) -> np.ndarray:
    out, _ = run(inputs, trace=False)
    return out
